# revision 43
# baseline (speedup 1.0000x reference)
"""EoMT criterion (Mask2Former-style loss) on 8 Trainium2 NeuronCores.

Math reduction: for each image with mask logits x [N=100, HW] and binary gt
masks y [M=20, HW], every term of the loss only needs
    A  = x @ y.T                  (since log p - log(1-p) = x)
    B  = sigmoid(x) @ y.T
    sp = sum_px softplus(x)  per row      (= -rowsum log(1-p))
    ps = sum_px sigmoid(x)   per row
    ys = sum_px y            per row (host, exact ints)
because
    bce_cost[n,m]  = (sp[n] - A[n,m]) / HW
    dice_cost[n,m] = 1 - (2 B[n,m] + 1) / (ps[n] + ys[m] + 1)
    matched-pair bce  = sum_k (sp[pi_k] - A[pi_k, gi_k]) / (K*HW)
    matched-pair dice from B/ps/ys at the matched indices.
The device reduces 250MB of inputs to one [21, 300] f32 tile per core; the
Hungarian assignment and the tiny class-logit terms run on host.

Sharding: 8 cores = 2 images x 4 HW-quarters. Host pre-transposes x to
pixel-major [HW, N] bf16 so the pixel (contraction) axis lands on SBUF
partitions, and appends a ones column to y ([HW, 21]) so row 20 of each
matmul output carries the per-row pixel sums (ps, sp).
"""

import numpy as np
import ml_dtypes

BF16 = ml_dtypes.bfloat16

N_CORES = 8
BS = 2
NQ = 100          # number of mask queries
NM = 20           # number of gt masks
NC1 = 7           # classes + no-object
H = W = 512
HW = H * W                      # 262144
PX_PER_CORE = HW // 4           # 65536
N_CHUNKS = 8
PX_PER_CHUNK = PX_PER_CORE // N_CHUNKS   # 8192
F = PX_PER_CHUNK // 128                  # 64 pixel-groups per chunk
NGROUP = PX_PER_CORE // 128              # 512 groups per core

CLS_W, MASK_W, DICE_W, NO_OBJ_W = 2.0, 5.0, 5.0, 0.1
NO_OBJ = 6

_NC_CACHE = {}


def _build_device_kernel(reps=1, variant="full"):
    """One SPMD program per core: inputs xt [65536,100] bf16 (pixel-major
    mask logits), yt [65536,21] bf16 (gt masks + ones col); output
    [21, 300] f32 = [A.T | Q.T | T.T] blocks where A = x @ y1.T,
    Q = sigmoid(-x) @ y1.T, T = ln(sigmoid(-x)) @ y1.T (row 20 = colsums).

    The compiler's activation tables have no softplus, so softplus/sigmoid
    come from q = sigmoid(-x): sigmoid(x) = 1-q, softplus(x) = -ln(q)."""
    import concourse.bacc as bacc
    import concourse.mybir as mybir
    import concourse.tile as tile

    nc = bacc.Bacc("TRN2", target_bir_lowering=False, debug=False,
                   num_devices=N_CORES)
    x_dt = mybir.dt.float8e4 if variant == "fp8" else mybir.dt.bfloat16
    x_d = nc.dram_tensor("xt", (PX_PER_CORE, NQ), x_dt,
                         kind="ExternalInput")
    y_d = nc.dram_tensor("yt", (PX_PER_CORE, NM + 1), mybir.dt.bfloat16,
                         kind="ExternalInput")
    y8_d = None
    if variant == "fp8":
        y8_d = nc.dram_tensor("yt8", (PX_PER_CORE, NM + 1),
                              mybir.dt.float8e4, kind="ExternalInput")
    out_rows = 128 if variant == "full5" else NM + 1
    out_d = nc.dram_tensor("out_res", (out_rows, 3 * NQ), mybir.dt.float32,
                           kind="ExternalOutput")
    if variant == "fp8":
        _build_body_fp8(nc, x_d, y_d, y8_d, out_d, reps)
        nc.compile()
        return nc

    AF = mybir.ActivationFunctionType
    bf16 = mybir.dt.bfloat16
    f32 = mybir.dt.float32

    if variant == "full6":
        _build_body6(nc, x_d, y_d, out_d, reps)
        nc.compile()
        return nc

    if variant == "peonly":
        # pure PE chain: the full 1536-matmul schedule from static tiles
        bf16_ = mybir.dt.bfloat16
        f32_ = mybir.dt.float32
        with tile.TileContext(nc) as tc:
            with (
                tc.tile_pool(name="sp", bufs=1) as sp_,
                tc.tile_pool(name="pp", bufs=1, space="PSUM") as pp,
            ):
                y_s = sp_.tile([128, NM + 1], bf16_)
                x_s = sp_.tile([128, F, NQ], bf16_)
                q_s = sp_.tile([128, F, NQ], bf16_)
                t_s = sp_.tile([128, F, NQ], bf16_)
                for t in (y_s, x_s, q_s, t_s):
                    nc.gpsimd.memset(t[:], 0.0)
                psA_ = pp.tile([NM + 1, NQ], f32_)
                psQ_ = pp.tile([NM + 1, NQ], f32_)
                psT_ = pp.tile([NM + 1, NQ], f32_)

                def body_pe():
                    for g in range(NGROUP):
                        f = g % F
                        st = g == 0
                        spf = g == NGROUP - 1
                        nc.tensor.matmul(psA_[:], y_s[:], x_s[:, f, :],
                                         start=st, stop=spf)
                        nc.tensor.matmul(psQ_[:], y_s[:], q_s[:, f, :],
                                         start=st, stop=spf)
                        nc.tensor.matmul(psT_[:], y_s[:], t_s[:, f, :],
                                         start=st, stop=spf)

                if reps == 1:
                    body_pe()
                else:
                    with tc.For_i(0, reps, 1):
                        body_pe()
                out_sb = sp_.tile([NM + 1, 3 * NQ], f32_)
                nc.vector.tensor_copy(out_sb[:, 0:NQ], psA_[:])
                nc.vector.tensor_copy(out_sb[:, NQ:2 * NQ], psQ_[:])
                nc.vector.tensor_copy(out_sb[:, 2 * NQ:3 * NQ], psT_[:])
                nc.sync.dma_start(out_d.ap(), out_sb[:])
        nc.compile()
        return nc

    if variant == "empty":
        # calibration: measures pure For_i back-edge + barrier cost
        with tile.TileContext(nc) as tc:
            with tc.tile_pool(name="zp", bufs=1) as zp:
                z_t = zp.tile([128, 8], mybir.dt.float32)
                if reps == 1:
                    nc.gpsimd.memset(z_t[:], 0.0)
                else:
                    with tc.For_i(0, reps, 1):
                        nc.gpsimd.memset(z_t[:], 0.0)
                out_sb = zp.tile([NM + 1, 3 * NQ], mybir.dt.float32)
                nc.gpsimd.memset(out_sb[:], 0.0)
                nc.sync.dma_start(out_d.ap(), out_sb[:])
        nc.compile()
        return nc

    with tile.TileContext(nc) as tc:
        with (
            tc.tile_pool(name="xpool", bufs=3) as xpool,
            tc.tile_pool(name="qpool", bufs=1) as qpool,
            tc.tile_pool(name="ypool", bufs=1) as ypool,
            tc.tile_pool(name="tpool", bufs=2) as tpool,
            tc.tile_pool(name="opool", bufs=1) as opool,
            tc.tile_pool(name="pspool", bufs=1, space="PSUM") as pspool,
        ):
            y_tile = ypool.tile([128, N_CHUNKS, F, NM + 1], bf16)
            y_view = y_d.ap().rearrange("(c p f) j -> c p f j",
                                        c=N_CHUNKS, p=128, f=F)

            psA = pspool.tile([NM + 1, NQ], f32)
            psQ = pspool.tile([NM + 1, NQ], f32)
            psT = pspool.tile([NM + 1, NQ], f32)
            psAQ = None
            if variant == "full4":
                psAQ = pspool.tile([NM + 1, 2 * NQ], f32)
            psAQ5 = psT5 = None
            if variant == "full5":
                psAQ5 = pspool.tile([128, 2 * NQ], f32)
                psT5 = pspool.tile([128, NQ], f32)

            x_view = x_d.ap().rearrange("(c p f) j -> c p f j",
                                        c=N_CHUNKS, p=128, f=F)
            q_tiles = [qpool.tile([128, F, NQ], bf16, name=f"q_{c}")
                       for c in range(N_CHUNKS)]
            if variant == "noact":
                for q_t in q_tiles:
                    nc.gpsimd.memset(q_t[:], 0.0)
            x_static = None
            if variant in ("actonly", "actonly2"):
                x_static = xpool.tile([128, F, NQ], bf16, name="x_static",
                                      tag="xs")
                nc.gpsimd.memset(x_static[:], 0.0)

            def emit_act_only(nsplit):
                # pure ACT chain: sigma then ln over the same volumes, no DMA
                sub = F // nsplit
                for c in range(N_CHUNKS):
                    q_t = q_tiles[c]
                    for s in range(nsplit):
                        sl = slice(s * sub, (s + 1) * sub)
                        nc.scalar.activation(q_t[:, sl, :],
                                             x_static[:, sl, :],
                                             AF.Sigmoid, scale=-1.0)
                for c in range(N_CHUNKS):
                    q_t = q_tiles[c]
                    t_t = tpool.tile([128, F, NQ], bf16, name="t_t", tag="t")
                    for s in range(nsplit):
                        sl = slice(s * sub, (s + 1) * sub)
                        nc.scalar.activation(t_t[:, sl, :], q_t[:, sl, :],
                                             AF.Ln)

            def emit_body4():
                # phase 1 with merged A|Q matmuls: one N=200 matmul per
                # group over the combined [x | q] transient tile.
                for c in range(N_CHUNKS):
                    xq_t = xpool.tile([128, 2, F, NQ], bf16, name="xq_t",
                                      tag="xq", bufs=2)
                    q_t = q_tiles[c]
                    splits = 4 if c == 0 else 1
                    sub = F // splits
                    for s in range(splits):
                        sl = slice(s * sub, (s + 1) * sub)
                        nc.sync.dma_start(xq_t[:, 0, sl, :],
                                          x_view[c][:, sl, :])
                        nc.sync.dma_start(y_tile[:, c, sl, :],
                                          y_view[c][:, sl, :])
                        nc.scalar.activation(q_t[:, sl, :], xq_t[:, 0, sl, :],
                                             AF.Sigmoid, scale=-1.0)
                        nc.vector.tensor_copy(xq_t[:, 1, sl, :], q_t[:, sl, :])
                        for f in range(s * sub, (s + 1) * sub):
                            g = c * F + f
                            nc.tensor.matmul(psAQ[:], y_tile[:, c, f, :],
                                             xq_t[:, :, f, :],
                                             start=(g == 0),
                                             stop=(g == NGROUP - 1))
                # phase 2 unchanged
                for c in range(N_CHUNKS):
                    q_t = q_tiles[c]
                    t_t = tpool.tile([128, F, NQ], bf16, name="t_t", tag="t")
                    splits = 4 if c == N_CHUNKS - 1 else 1
                    sub = F // splits
                    for s in range(splits):
                        sl = slice(s * sub, (s + 1) * sub)
                        nc.scalar.activation(t_t[:, sl, :], q_t[:, sl, :],
                                             AF.Ln)
                        for f in range(s * sub, (s + 1) * sub):
                            g = c * F + f
                            nc.tensor.matmul(psT[:], y_tile[:, c, f, :],
                                             t_t[:, f, :], start=(g == 0),
                                             stop=(g == NGROUP - 1))

            def emit_body5():
                # col-tiled matmuls: group g runs in PE column strip g%4 via
                # tile_position, so 4 groups execute concurrently. Each strip
                # has its own accumulation chain in rows [32j, 32j+21) of a
                # shared PSUM tile; host sums the strips.
                NS = 4
                for c in range(N_CHUNKS):
                    x_t = xpool.tile([128, F, NQ], bf16, name="x_t", tag="x")
                    q_t = q_tiles[c]
                    splits = 4 if c == 0 else 1
                    sub = F // splits
                    for s in range(splits):
                        sl = slice(s * sub, (s + 1) * sub)
                        nc.sync.dma_start(x_t[:, sl, :], x_view[c][:, sl, :])
                        nc.sync.dma_start(y_tile[:, c, sl, :],
                                          y_view[c][:, sl, :])
                        nc.scalar.activation(q_t[:, sl, :], x_t[:, sl, :],
                                             AF.Sigmoid, scale=-1.0)
                        for f in range(s * sub, (s + 1) * sub):
                            g = c * F + f
                            j = g % NS
                            st = g < NS
                            sp = g >= NGROUP - NS
                            r = slice(32 * j, 32 * j + NM + 1)
                            nc.tensor.matmul(psAQ5[r, 0:NQ],
                                             y_tile[:, c, f, :],
                                             x_t[:, f, :], start=st, stop=sp,
                                             tile_position=(0, 32 * j),
                                             skip_group_check=True)
                            nc.tensor.matmul(psAQ5[r, NQ:2 * NQ],
                                             y_tile[:, c, f, :],
                                             q_t[:, f, :], start=st, stop=sp,
                                             tile_position=(0, 32 * j),
                                             skip_group_check=True)
                for c in range(N_CHUNKS):
                    q_t = q_tiles[c]
                    t_t = tpool.tile([128, F, NQ], bf16, name="t_t", tag="t")
                    splits = 4 if c == N_CHUNKS - 1 else 1
                    sub = F // splits
                    for s in range(splits):
                        sl = slice(s * sub, (s + 1) * sub)
                        nc.scalar.activation(t_t[:, sl, :], q_t[:, sl, :],
                                             AF.Ln)
                        for f in range(s * sub, (s + 1) * sub):
                            g = c * F + f
                            j = g % NS
                            st = g < NS
                            sp = g >= NGROUP - NS
                            r = slice(32 * j, 32 * j + NM + 1)
                            nc.tensor.matmul(psT5[r, :], y_tile[:, c, f, :],
                                             t_t[:, f, :], start=st, stop=sp,
                                             tile_position=(0, 32 * j),
                                             skip_group_check=True)

            def emit_body9():
                # y loaded as ONE upfront DMA on the ACT HWDGE ring (no deps,
                # fires immediately, keeps the sync ring exclusive to x)
                nc.scalar.dma_start(
                    y_tile[:],
                    y_d.ap().rearrange("(c p f) j -> p c f j",
                                       c=N_CHUNKS, p=128, f=F))
                for c in range(N_CHUNKS):
                    x_t = xpool.tile([128, F, NQ], bf16, name="x_t", tag="x")
                    q_t = q_tiles[c]
                    splits = 4 if c == 0 else 1
                    sub = F // splits
                    for s in range(splits):
                        sl = slice(s * sub, (s + 1) * sub)
                        nc.sync.dma_start(x_t[:, sl, :], x_view[c][:, sl, :])
                        nc.scalar.activation(q_t[:, sl, :], x_t[:, sl, :],
                                             AF.Sigmoid, scale=-1.0)
                        for f in range(s * sub, (s + 1) * sub):
                            g = c * F + f
                            st = g == 0
                            sp = g == NGROUP - 1
                            nc.tensor.matmul(psA[:], y_tile[:, c, f, :],
                                             x_t[:, f, :], start=st, stop=sp)
                            nc.tensor.matmul(psQ[:], y_tile[:, c, f, :],
                                             q_t[:, f, :], start=st, stop=sp)
                for c in range(N_CHUNKS):
                    q_t = q_tiles[c]
                    t_t = tpool.tile([128, F, NQ], bf16, name="t_t", tag="t")
                    splits = 4 if c == N_CHUNKS - 1 else 1
                    sub = F // splits
                    for s in range(splits):
                        sl = slice(s * sub, (s + 1) * sub)
                        nc.scalar.activation(t_t[:, sl, :], q_t[:, sl, :],
                                             AF.Ln)
                        for f in range(s * sub, (s + 1) * sub):
                            g = c * F + f
                            st = g == 0
                            sp = g == NGROUP - 1
                            nc.tensor.matmul(psT[:], y_tile[:, c, f, :],
                                             t_t[:, f, :], start=st, stop=sp)

            def emit_body():
                if variant == "actonly":
                    return emit_act_only(1)
                if variant == "actonly2":
                    return emit_act_only(2)
                if variant == "full4":
                    return emit_body4()
                if variant == "full5":
                    return emit_body5()
                if variant == "full9":
                    return emit_body9()
                do_mm = variant in ("full", "noact", "full2", "full3")
                do_act = variant in ("full", "nomm", "full2", "full3")
                # two-ring DMA: x chunks alternate between the SP and ACT
                # HWDGE rings; y goes on the ACT ring. The *3 variants use
                # the idle GPSIMD engine's SWDGE path as the second ring so
                # the ACT compute stream is never blocked by DMA triggers.
                two_ring = variant in ("dmaonly2", "full2")
                swdge = variant in ("dmaonly3", "full3")

                def x_dma(c, dst, src):
                    if two_ring and c % 2 == 1:
                        eng = nc.scalar
                    elif swdge and c % 2 == 1:
                        eng = nc.gpsimd
                    else:
                        eng = nc.sync
                    eng.dma_start(dst, src)

                def y_dma(dst, src):
                    if two_ring:
                        eng = nc.scalar
                    elif swdge:
                        eng = nc.gpsimd
                    else:
                        eng = nc.sync
                    eng.dma_start(dst, src)
                # phase 1: q = sigmoid(-x); accumulate A (raw x) and Q (q).
                # Chunk 0 is emitted in slivers so the ACT engine starts as
                # soon as the first slice of x lands.
                for c in range(N_CHUNKS):
                    x_t = xpool.tile([128, F, NQ], bf16, name="x_t", tag="x")
                    q_t = q_tiles[c]
                    splits = 4 if c == 0 else 1
                    sub = F // splits
                    for s in range(splits):
                        sl = slice(s * sub, (s + 1) * sub)
                        x_dma(c, x_t[:, sl, :], x_view[c][:, sl, :])
                        y_dma(y_tile[:, c, sl, :], y_view[c][:, sl, :])
                        if do_act:
                            nc.scalar.activation(q_t[:, sl, :], x_t[:, sl, :],
                                                 AF.Sigmoid, scale=-1.0)
                        if do_mm:
                            src1 = q_t if do_act else x_t
                            for f in range(s * sub, (s + 1) * sub):
                                g = c * F + f
                                st = g == 0
                                sp = g == NGROUP - 1
                                nc.tensor.matmul(psA[:], y_tile[:, c, f, :],
                                                 x_t[:, f, :],
                                                 start=st, stop=sp)
                                nc.tensor.matmul(psQ[:], y_tile[:, c, f, :],
                                                 src1[:, f, :],
                                                 start=st, stop=sp)
                # phase 2: t = ln(q) = -softplus(x); accumulate T.
                # The last chunk is emitted in slivers to shrink the tail.
                for c in range(N_CHUNKS):
                    q_t = q_tiles[c]
                    t_t = tpool.tile([128, F, NQ], bf16, name="t_t", tag="t")
                    splits = 4 if c == N_CHUNKS - 1 else 1
                    sub = F // splits
                    for s in range(splits):
                        sl = slice(s * sub, (s + 1) * sub)
                        if do_act:
                            nc.scalar.activation(t_t[:, sl, :], q_t[:, sl, :],
                                                 AF.Ln)
                        if do_mm:
                            src2 = t_t if do_act else q_t
                            for f in range(s * sub, (s + 1) * sub):
                                g = c * F + f
                                st = g == 0
                                sp = g == NGROUP - 1
                                nc.tensor.matmul(psT[:], y_tile[:, c, f, :],
                                                 src2[:, f, :],
                                                 start=st, stop=sp)

            if reps == 1:
                emit_body()
            else:
                # timing mode: repeat the full body (DMA + ACT + PE) inside
                # the NEFF; every iteration recomputes from scratch (start=
                # True clears PSUM), so the final output is still correct.
                with tc.For_i(0, reps, 1):
                    emit_body()

            out_sb = opool.tile([out_rows, 3 * NQ], f32)
            if variant == "full5":
                nc.vector.tensor_copy(out_sb[:, 0:2 * NQ], psAQ5[:])
                nc.vector.tensor_copy(out_sb[:, 2 * NQ:3 * NQ], psT5[:])
            elif variant == "full4":
                nc.vector.tensor_copy(out_sb[:, 0:2 * NQ], psAQ[:])
                nc.vector.tensor_copy(out_sb[:, 2 * NQ:3 * NQ], psT[:])
            elif variant in ("full", "noact", "full2", "full3", "full9"):
                nc.vector.tensor_copy(out_sb[:, 0:NQ], psA[:])
                nc.vector.tensor_copy(out_sb[:, NQ:2 * NQ], psQ[:])
                nc.vector.tensor_copy(out_sb[:, 2 * NQ:3 * NQ], psT[:])
            else:
                nc.gpsimd.memset(out_sb[:], 0.0)
            nc.sync.dma_start(out_d.ap(), out_sb[:])

    nc.compile()
    return nc


def _build_body_fp8(nc, x_d, y_d, y8_d, out_d, reps):
    """fp8 x: halves the dominant x DMA stream. The A-matmul runs fully in
    fp8 (x rhs, fp8 y copy as lhsT); sigma reads fp8 x and produces bf16 q,
    so the Q/T (ln) path keeps bf16 precision. Q-matmuls of the last 4
    chunks are deferred to phase 2 to balance PE between phases."""
    import concourse.mybir as mybir
    import concourse.tile as tile

    AF = mybir.ActivationFunctionType
    bf16 = mybir.dt.bfloat16
    fp8 = mybir.dt.float8e4
    f32 = mybir.dt.float32

    with tile.TileContext(nc) as tc:
        with (
            tc.tile_pool(name="xpool", bufs=3) as xpool,
            tc.tile_pool(name="qpool", bufs=1) as qpool,
            tc.tile_pool(name="ypool", bufs=1) as ypool,
            tc.tile_pool(name="tpool", bufs=2) as tpool,
            tc.tile_pool(name="opool", bufs=1) as opool,
            tc.tile_pool(name="pspool", bufs=1, space="PSUM") as pspool,
        ):
            y_tile = ypool.tile([128, N_CHUNKS, F, NM + 1], bf16)
            y8_tile = ypool.tile([128, N_CHUNKS, F, NM + 1], fp8)
            y_view = y_d.ap().rearrange("(c p f) j -> c p f j",
                                        c=N_CHUNKS, p=128, f=F)
            y8_view = y8_d.ap().rearrange("(c p f) j -> c p f j",
                                          c=N_CHUNKS, p=128, f=F)
            x_view = x_d.ap().rearrange("(c p f) j -> c p f j",
                                        c=N_CHUNKS, p=128, f=F)
            psA = pspool.tile([NM + 1, NQ], f32)
            psQ = pspool.tile([NM + 1, NQ], f32)
            psT = pspool.tile([NM + 1, NQ], f32)
            q_tiles = [qpool.tile([128, F, NQ], bf16, name=f"q_{c}")
                       for c in range(N_CHUNKS)]

            def emit_body():
                half = N_CHUNKS // 2
                for c in range(N_CHUNKS):
                    x_t = xpool.tile([128, F, NQ], fp8, name="x_t", tag="x")
                    q_t = q_tiles[c]
                    splits = 4 if c == 0 else 1
                    sub = F // splits
                    for s in range(splits):
                        sl = slice(s * sub, (s + 1) * sub)
                        nc.sync.dma_start(x_t[:, sl, :], x_view[c][:, sl, :])
                        nc.sync.dma_start(y8_tile[:, c, sl, :],
                                          y8_view[c][:, sl, :])
                        nc.sync.dma_start(y_tile[:, c, sl, :],
                                          y_view[c][:, sl, :])
                        nc.scalar.activation(q_t[:, sl, :], x_t[:, sl, :],
                                             AF.Sigmoid, scale=-1.0)
                        for f in range(s * sub, (s + 1) * sub):
                            g = c * F + f
                            st = g == 0
                            sp = g == NGROUP - 1
                            nc.tensor.matmul(psA[:], y8_tile[:, c, f, :],
                                             x_t[:, f, :], start=st, stop=sp)
                            if c < half:
                                nc.tensor.matmul(psQ[:], y_tile[:, c, f, :],
                                                 q_t[:, f, :],
                                                 start=st, stop=False)
                for c in range(N_CHUNKS):
                    q_t = q_tiles[c]
                    t_t = tpool.tile([128, F, NQ], bf16, name="t_t", tag="t")
                    splits = 4 if c == N_CHUNKS - 1 else 1
                    sub = F // splits
                    for s in range(splits):
                        sl = slice(s * sub, (s + 1) * sub)
                        nc.scalar.activation(t_t[:, sl, :], q_t[:, sl, :],
                                             AF.Ln)
                        for f in range(s * sub, (s + 1) * sub):
                            g = c * F + f
                            st = g == 0
                            sp = g == NGROUP - 1
                            nc.tensor.matmul(psT[:], y_tile[:, c, f, :],
                                             t_t[:, f, :], start=st, stop=sp)
                            if c >= half:
                                gq = (c - half) * F + f
                                nc.tensor.matmul(psQ[:], y_tile[:, c, f, :],
                                                 q_t[:, f, :], start=False,
                                                 stop=(gq == NGROUP // 2 - 1))

            if reps == 1:
                emit_body()
            else:
                with tc.For_i(0, reps, 1):
                    emit_body()

            out_sb = opool.tile([NM + 1, 3 * NQ], f32)
            nc.vector.tensor_copy(out_sb[:, 0:NQ], psA[:])
            nc.vector.tensor_copy(out_sb[:, NQ:2 * NQ], psQ[:])
            nc.vector.tensor_copy(out_sb[:, 2 * NQ:3 * NQ], psT[:])
            nc.sync.dma_start(out_d.ap(), out_sb[:])


def _build_body6(nc, x_d, y_d, out_d, reps):
    """Rebalanced pipeline: 4 chunks of 16384 px (3.3MB x-DMAs, FD=12800
    ACT calls); Q-matmuls of the last 2 chunks are deferred to phase 2 so
    PE work is split ~evenly between the DMA-bound sigma phase and the
    ACT-bound ln phase."""
    import concourse.mybir as mybir
    import concourse.tile as tile

    AF = mybir.ActivationFunctionType
    bf16 = mybir.dt.bfloat16
    f32 = mybir.dt.float32
    C6 = 4
    F6 = PX_PER_CORE // (128 * C6)   # 128 groups per chunk

    with tile.TileContext(nc) as tc:
        with (
            tc.tile_pool(name="xpool", bufs=2) as xpool,
            tc.tile_pool(name="qpool", bufs=1) as qpool,
            tc.tile_pool(name="ypool", bufs=1) as ypool,
            tc.tile_pool(name="tpool", bufs=1) as tpool,
            tc.tile_pool(name="opool", bufs=1) as opool,
            tc.tile_pool(name="pspool", bufs=1, space="PSUM") as pspool,
        ):
            y_tile = ypool.tile([128, C6, F6, NM + 1], bf16)
            y_view = y_d.ap().rearrange("(c p f) j -> c p f j",
                                        c=C6, p=128, f=F6)
            x_view = x_d.ap().rearrange("(c p f) j -> c p f j",
                                        c=C6, p=128, f=F6)
            psA = pspool.tile([NM + 1, NQ], f32)
            psQ = pspool.tile([NM + 1, NQ], f32)
            psT = pspool.tile([NM + 1, NQ], f32)
            q_tiles = [qpool.tile([128, F6, NQ], bf16, name=f"q_{c}")
                       for c in range(C6)]

            def emit_body():
                # phase 1: DMA + sigma; A-matmuls for all chunks, Q-matmuls
                # only for the first half of chunks
                for c in range(C6):
                    x_t = xpool.tile([128, F6, NQ], bf16, name="x_t", tag="x")
                    q_t = q_tiles[c]
                    splits = 4 if c == 0 else 1
                    sub = F6 // splits
                    for s in range(splits):
                        sl = slice(s * sub, (s + 1) * sub)
                        nc.sync.dma_start(x_t[:, sl, :], x_view[c][:, sl, :])
                        nc.sync.dma_start(y_tile[:, c, sl, :],
                                          y_view[c][:, sl, :])
                        nc.scalar.activation(q_t[:, sl, :], x_t[:, sl, :],
                                             AF.Sigmoid, scale=-1.0)
                        for f in range(s * sub, (s + 1) * sub):
                            g = c * F6 + f
                            st = g == 0
                            sp = g == NGROUP - 1
                            nc.tensor.matmul(psA[:], y_tile[:, c, f, :],
                                             x_t[:, f, :], start=st, stop=sp)
                            if c < C6 // 2:
                                nc.tensor.matmul(psQ[:], y_tile[:, c, f, :],
                                                 q_t[:, f, :],
                                                 start=st, stop=False)
                # phase 2: ln + T-matmuls + deferred Q-matmuls
                for c in range(C6):
                    q_t = q_tiles[c]
                    t_t = tpool.tile([128, F6, NQ], bf16, name="t_t", tag="t")
                    splits = 4 if c == C6 - 1 else 1
                    sub = F6 // splits
                    for s in range(splits):
                        sl = slice(s * sub, (s + 1) * sub)
                        nc.scalar.activation(t_t[:, sl, :], q_t[:, sl, :],
                                             AF.Ln)
                        for f in range(s * sub, (s + 1) * sub):
                            g = c * F6 + f
                            st = g == 0
                            sp = g == NGROUP - 1
                            nc.tensor.matmul(psT[:], y_tile[:, c, f, :],
                                             t_t[:, f, :], start=st, stop=sp)
                            if c >= C6 // 2:
                                gq = (c - C6 // 2) * F6 + f
                                nc.tensor.matmul(psQ[:], y_tile[:, c, f, :],
                                                 q_t[:, f, :], start=False,
                                                 stop=(gq == NGROUP // 2 - 1))

            if reps == 1:
                emit_body()
            else:
                with tc.For_i(0, reps, 1):
                    emit_body()

            out_sb = opool.tile([NM + 1, 3 * NQ], f32)
            nc.vector.tensor_copy(out_sb[:, 0:NQ], psA[:])
            nc.vector.tensor_copy(out_sb[:, NQ:2 * NQ], psQ[:])
            nc.vector.tensor_copy(out_sb[:, 2 * NQ:3 * NQ], psT[:])
            nc.sync.dma_start(out_d.ap(), out_sb[:])


def _get_nc(reps=1, variant="full"):
    key = (reps, variant)
    if key not in _NC_CACHE:
        _NC_CACHE[key] = _build_device_kernel(reps, variant)
    return _NC_CACHE[key]


def _prepare_in_maps(mask_logits, gt_masks, fp8=False):
    """Host-side marshalling: transpose to pixel-major, cast, shard."""
    FP8 = ml_dtypes.float8_e4m3
    m2 = mask_logits.reshape(BS, NQ, HW)
    g2 = gt_masks.reshape(BS, NM, HW)
    in_maps = []
    for b in range(BS):
        for q in range(4):
            sl = slice(q * PX_PER_CORE, (q + 1) * PX_PER_CORE)
            xt = np.ascontiguousarray(m2[b, :, sl].T)
            yt = np.empty((PX_PER_CORE, NM + 1), dtype=BF16)
            yt[:, :NM] = g2[b, :, sl].T
            yt[:, NM] = BF16(1.0)
            m = {"xt": xt.astype(FP8 if fp8 else BF16), "yt": yt}
            if fp8:
                m["yt8"] = yt.astype(FP8)
            in_maps.append(m)
    return in_maps


def _run_device(in_maps, reps=1, variant="full", trace=False):
    from concourse import bass_utils
    nc = _get_nc(reps, variant)
    res = bass_utils.run_bass_kernel_spmd(
        nc, in_maps, core_ids=list(range(N_CORES)), trace=trace)
    return res


def _hungarian(cost):
    """Jonker-Volgenant shortest augmenting path; equivalent to scipy's
    linear_sum_assignment. cost [n, m] -> (row_ind, col_ind) sorted by row."""
    cost = np.asarray(cost, dtype=np.float64)
    transposed = cost.shape[0] > cost.shape[1]
    if transposed:
        cost = cost.T
    n, m = cost.shape
    INF = 1e18
    u = np.zeros(n + 1)
    v = np.zeros(m + 1)
    p = np.zeros(m + 1, dtype=np.int64)
    way = np.zeros(m + 1, dtype=np.int64)
    for i in range(1, n + 1):
        p[0] = i
        j0 = 0
        minv = np.full(m + 1, INF)
        used = np.zeros(m + 1, dtype=bool)
        while True:
            used[j0] = True
            i0 = p[j0]
            cand = cost[i0 - 1] - u[i0] - v[1:]
            upd = (~used[1:]) & (cand < minv[1:])
            minv[1:] = np.where(upd, cand, minv[1:])
            way[1:] = np.where(upd, j0, way[1:])
            masked = np.where(used[1:], INF, minv[1:])
            j1 = int(np.argmin(masked)) + 1
            delta = masked[j1 - 1]
            u[p[used]] += delta
            v[used] -= delta
            minv[1:][~used[1:]] -= delta
            j0 = j1
            if p[j0] == 0:
                break
        while j0:
            j1 = way[j0]
            p[j0] = p[j1]
            j0 = j1
    rows, cols = [], []
    for j in range(1, m + 1):
        if p[j] != 0:
            rows.append(p[j] - 1)
            cols.append(j - 1)
    rows = np.asarray(rows, dtype=np.int64)
    cols = np.asarray(cols, dtype=np.int64)
    if transposed:
        rows, cols = cols, rows
    order = np.argsort(rows)
    return rows[order], cols[order]


def _finish_on_host(core_outs, class_logits, gt_classes, gt_masks):
    """Combine per-core [21, 300] partials, assemble costs, match, and
    compute the four loss scalars."""
    g2 = gt_masks.reshape(BS, NM, HW)
    cls64 = class_logits.astype(np.float64)

    tc = tm = td = 0.0
    for b in range(BS):
        tot = np.zeros((NM + 1, 3 * NQ), dtype=np.float64)
        for q in range(4):
            o = core_outs[4 * b + q].astype(np.float64)
            if o.shape[0] == 128:   # col-tiled strips: sum rows 32j..32j+21
                o = sum(o[32 * j:32 * j + NM + 1] for j in range(4))
            tot += o
        ys = g2[b].sum(axis=1).astype(np.float64)   # [M]
        A = tot[:NM, 0:NQ].T                 # [N, M] = x @ y.T
        Q = tot[:NM, NQ:2 * NQ].T            # [N, M] = sigmoid(-x) @ y.T
        qs = tot[NM, NQ:2 * NQ]              # [N]    = rowsum sigmoid(-x)
        B = ys[None, :] - Q                  # [N, M] = sigmoid(x) @ y.T
        ps = HW - qs                         # [N]    = rowsum sigmoid(x)
        sp = -tot[NM, 2 * NQ:3 * NQ]         # [N]    = rowsum softplus(x)

        # cost matrix
        cl = cls64[b]                        # [N, 7]
        z = cl - cl.max(axis=1, keepdims=True)
        ez = np.exp(z)
        prob = ez / ez.sum(axis=1, keepdims=True)
        gt_cls = gt_classes[b].astype(np.int64)
        class_cost = -prob[:, gt_cls]                       # [N, M]
        bce_cost = (sp[:, None] - A) / HW
        dice_cost = 1.0 - (2.0 * B + 1.0) / (ps[:, None] + ys[None, :] + 1.0)
        cost = CLS_W * class_cost + MASK_W * bce_cost + DICE_W * dice_cost

        pi, gi = _hungarian(cost)

        # classification loss (weighted-mean CE, torch semantics)
        logp = z - np.log(ez.sum(axis=1, keepdims=True))
        target = np.full(NQ, NO_OBJ, dtype=np.int64)
        target[pi] = gt_cls[gi]
        nll = -logp[np.arange(NQ), target]
        wts = np.where(target == NO_OBJ, NO_OBJ_W, 1.0)
        cls_loss = (wts * nll).sum() / wts.sum()

        # matched-pair mask bce + dice
        K = pi.shape[0]
        bce = (sp[pi] - A[pi, gi]).sum() / (K * HW)
        dice = (1.0 - (2.0 * B[pi, gi] + 1.0) / (ps[pi] + ys[gi] + 1.0)).mean()

        tc += cls_loss
        tm += bce
        td += dice

    tc, tm, td = tc / BS, tm / BS, td / BS
    total = CLS_W * tc + MASK_W * tm + DICE_W * td
    return np.array([tc, tm, td, total], dtype=np.float32)


PROD_VARIANT = "fp8"


def kernel(class_logits, mask_logits, gt_classes, gt_masks):
    class_logits = np.asarray(class_logits)
    mask_logits = np.asarray(mask_logits)
    gt_classes = np.asarray(gt_classes)
    gt_masks = np.asarray(gt_masks)

    in_maps = _prepare_in_maps(mask_logits, gt_masks,
                               fp8=(PROD_VARIANT == "fp8"))
    res = _run_device(in_maps, variant=PROD_VARIANT)
    core_outs = [r["out_res"] for r in res.results]
    return _finish_on_host(core_outs, class_logits, gt_classes, gt_masks)
